# revision 4
# baseline (speedup 1.0000x reference)
"""Trainium2 Bass kernel for nn_LossFunction_40346922778857.

Computes: scatter-loss over x (256,128,768).
  x1 = x[::2], x2 = x[1::2]  (each (128,128,768))
  per half: within (D,D), between (D,D) scatter matrices, corr-normalized,
  loss = sum((w1-w2)^2) + sum((b1-b2)^2).

Strategy (data-parallel over b across 8 cores):
  within = (G - N * Xbar^T Xbar) / (B*N)   with G = X^T X over (B*N, D)
  between = N * (Xbar^T Xbar - B mean mean^T) / (B*N)
  Each core computes partial G (upper-triangle 128-row blocks, fp16 inputs,
  fp32 PSUM accumulation) for its 16 even + 16 odd b's.  Per-b row-sums S
  fall out of the same matmuls via 16 appended one-hot columns.
  Host sums the 8 partials and finishes the O(D^2) algebra.
"""

import numpy as np

P = 128          # partitions / rows per b
D = 768          # feature dim
NB = 16          # number of b's (tiles) per half per core
DA = D + NB      # augmented width (one-hot tile-index columns)
L = 4            # k-tiles per DMA quarter
NQ = NB // L     # quarters per half
NCORES = 8
NBLK = D // P    # 6 row blocks of G

_STATE = {}
LAST = {}


def _build():
    import concourse.tile as tile
    from concourse import bacc, mybir
    from concourse.tile import add_dep_helper

    nc = bacc.Bacc("TRN2", target_bir_lowering=False, debug=False,
                   num_devices=NCORES)

    xins = [nc.dram_tensor(f"x{h}", [NQ, P, L * DA], mybir.dt.float16,
                           kind="ExternalInput").ap() for h in range(2)]
    outs = [nc.dram_tensor(f"o{h}", [D, DA], mybir.dt.float32,
                           kind="ExternalOutput").ap() for h in range(2)]

    with tile.TileContext(nc) as tc:
        with tc.tile_pool(name="xp", bufs=2 * NQ) as xp, \
             tc.tile_pool(name="wp", bufs=1) as wp, \
             tc.tile_pool(name="pp", bufs=4, space="PSUM") as pp, \
             tc.tile_pool(name="wpp", bufs=1, space="PSUM") as wpp, \
             tc.tile_pool(name="op", bufs=3) as op:
            # PE warm-up: ~4.5us of dummy matmuls while input DMAs stream,
            # so the HAM clock gate is at 8/8 when real matmuls start.
            wt = wp.tile([P, P], mybir.dt.float16, tag="wt")
            nc.vector.memset(wt[:], 0.0)
            wps = wpp.tile([P, P], mybir.dt.float32, tag="wps")
            for _ in range(12):
                nc.tensor.matmul(wps[:], wt[:], wt[:], start=True, stop=True)

            # Input DMAs, chained depth-2 so tiles arrive in consumption
            # order at full bandwidth instead of 8-way fair sharing.
            dma_chain = []
            all_q_tiles = [[], []]
            for h in range(2):
                xin = xins[h]
                for q in range(NQ):
                    xt = xp.tile([P, L * DA], mybir.dt.float16, tag="xt",
                                 name=f"x{h}q{q}")
                    d = nc.sync.dma_start(out=xt[:], in_=xin[q])
                    if dma_chain:
                        add_dep_helper(d.ins, dma_chain[-1].ins,
                                       reason="input dma ordering")
                    dma_chain.append(d)
                    all_q_tiles[h].append(xt)

            for h in range(2):
                oout = outs[h]
                q_tiles = all_q_tiles[h]
                for i in range(NBLK):
                    w_all = DA - P * i
                    chunks = []
                    off = 0
                    while off < w_all:
                        w = min(512, w_all - off)
                        chunks.append((off, w))
                        off += w
                    pts = [pp.tile([P, 512], mybir.dt.float32, tag="ps",
                                   name=f"ps{h}b{i}c{ci}")
                           for ci in range(len(chunks))]
                    for t in range(NB):
                        q, l = divmod(t, L)
                        xt = q_tiles[q]
                        base = l * DA + P * i
                        lhsT = xt[:, base:base + P]
                        for (off, w), pt in zip(chunks, pts):
                            nc.tensor.matmul(pt[:, :w], lhsT,
                                             xt[:, base + off:base + off + w],
                                             start=(t == 0), stop=(t == NB - 1))
                    ot = op.tile([P, w_all], mybir.dt.float32, tag="ot",
                                 name=f"o{h}b{i}")
                    for (off, w), pt in zip(chunks, pts):
                        nc.vector.tensor_copy(ot[:, off:off + w], pt[:, :w])
                    nc.sync.dma_start(out=oout[P * i:P * (i + 1), P * i:DA],
                                      in_=ot[:])
    nc.compile()
    return nc


def _get_nc():
    if "nc" not in _STATE:
        _STATE["nc"] = _build()
    return _STATE["nc"]


def _prep_half(xh):
    """xh: (128, 128, 768) f32 for one half -> per-core list of (NQ,P,L*DA) f16."""
    out = []
    for c in range(NCORES):
        blk = xh[NB * c:NB * (c + 1)]                      # (16, 128, 768)
        arr = np.zeros((NB, P, DA), dtype=np.float16)
        arr[:, :, :D] = blk
        for j in range(NB):
            arr[j, :, D + j] = 1.0
        # (t=4q+l, p, f) -> (q, p, l*DA+f)
        out.append(np.ascontiguousarray(
            arr.reshape(NQ, L, P, DA).transpose(0, 2, 1, 3).reshape(NQ, P, L * DA)))
    return out


def kernel(x, label=None, genre_label=None, _trace=False):
    from concourse.bass_utils import run_bass_kernel_spmd

    nc = _get_nc()

    x = np.asarray(x, dtype=np.float32)
    halves = [_prep_half(x[0::2]), _prep_half(x[1::2])]
    in_maps = [{"x0": halves[0][c], "x1": halves[1][c]} for c in range(NCORES)]

    res = run_bass_kernel_spmd(nc, in_maps, list(range(NCORES)), trace=_trace)
    LAST["res"] = res

    B = x.shape[0] // 2          # 128 b's per half
    N = x.shape[1]               # 128 rows per b
    tol = B * N

    loss = 0.0
    for h in range(2):
        U = np.zeros((D, D), dtype=np.float64)
        S = np.zeros((B, D), dtype=np.float64)
        for c in range(NCORES):
            o = np.asarray(res.results[c][f"o{h}"], dtype=np.float64)
            for i in range(NBLK):
                r = slice(P * i, P * (i + 1))
                U[r, P * i:D] += o[r, P * i:D]
            S[NB * c:NB * (c + 1)] += o[:, D:DA].T
        G = np.zeros((D, D), dtype=np.float64)
        for i in range(NBLK):
            ri = slice(P * i, P * (i + 1))
            G[ri, ri] = U[ri, ri]
            for j in range(i + 1, NBLK):
                rj = slice(P * j, P * (j + 1))
                G[ri, rj] = U[ri, rj]
                G[rj, ri] = U[ri, rj].T
        xbar = S / N
        M = xbar.T @ xbar
        mean = xbar.mean(axis=0)
        within = (G - N * M) / tol
        between = N * (M - B * np.outer(mean, mean)) / tol
        w_h = within / np.sqrt(np.sum(np.diagonal(within) ** 2))
        b_h = between / np.sqrt(np.sum(np.diagonal(between) ** 2))
        if h == 0:
            w0, b0 = w_h, b_h
        else:
            loss = np.sum((w0 - w_h) ** 2) + np.sum((b0 - b_h) ** 2)
    return np.asarray(loss, dtype=np.float32)


# revision 5
# speedup vs baseline: 1.0363x; 1.0363x over previous
"""Trainium2 Bass kernel for nn_LossFunction_40346922778857.

Computes: scatter-loss over x (256,128,768).
  x1 = x[::2], x2 = x[1::2]  (each (128,128,768))
  per half: within (D,D), between (D,D) scatter matrices, corr-normalized,
  loss = sum((w1-w2)^2) + sum((b1-b2)^2).

Strategy (data-parallel over b across 8 cores):
  within = (G - N * Xbar^T Xbar) / (B*N)   with G = X^T X over (B*N, D)
  between = N * (Xbar^T Xbar - B mean mean^T) / (B*N)
  Each core computes partial G (upper-triangle 128-row blocks, fp16 inputs,
  fp32 PSUM accumulation) for its 16 even + 16 odd b's.  Per-b row-sums S
  fall out of the same matmuls via 16 appended one-hot columns.
  Host sums the 8 partials and finishes the O(D^2) algebra.
"""

import numpy as np

P = 128          # partitions / rows per b
D = 768          # feature dim
NB = 16          # number of b's (tiles) per half per core
DA = D + NB      # augmented width (one-hot tile-index columns)
L = 4            # k-tiles per DMA quarter
NQ = NB // L     # quarters per half
NCORES = 8
NBLK = D // P    # 6 row blocks of G

_STATE = {}
LAST = {}


def _build():
    import concourse.tile as tile
    from concourse import bacc, mybir
    from concourse.tile import add_dep_helper

    nc = bacc.Bacc("TRN2", target_bir_lowering=False, debug=False,
                   num_devices=NCORES)

    xins = [nc.dram_tensor(f"x{h}", [NQ, P, L * DA], mybir.dt.float16,
                           kind="ExternalInput").ap() for h in range(2)]
    outs = [nc.dram_tensor(f"o{h}", [D, DA], mybir.dt.float32,
                           kind="ExternalOutput").ap() for h in range(2)]

    with tile.TileContext(nc) as tc:
        with tc.tile_pool(name="xp", bufs=2 * NQ) as xp, \
             tc.tile_pool(name="wp", bufs=1) as wp, \
             tc.tile_pool(name="pp", bufs=4, space="PSUM") as pp, \
             tc.tile_pool(name="wpp", bufs=1, space="PSUM") as wpp, \
             tc.tile_pool(name="op", bufs=3) as op:
            # PE warm-up: ~4.5us of dummy matmuls while input DMAs stream,
            # so the HAM clock gate is at 8/8 when real matmuls start.
            wt = wp.tile([P, P], mybir.dt.float16, tag="wt")
            nc.vector.memset(wt[:], 0.0)
            wps = wpp.tile([P, P], mybir.dt.float32, tag="wps")
            for _ in range(12):
                nc.tensor.matmul(wps[:], wt[:], wt[:], start=True, stop=True)

            # Input DMAs, chained depth-2 so tiles arrive in consumption
            # order at full bandwidth instead of 8-way fair sharing.
            dma_chain = []
            all_q_tiles = [[], []]
            for h in range(2):
                xin = xins[h]
                for q in range(NQ):
                    xt = xp.tile([P, L * DA], mybir.dt.float16, tag="xt",
                                 name=f"x{h}q{q}")
                    d = nc.sync.dma_start(out=xt[:], in_=xin[q])
                    if len(dma_chain) >= 2:
                        add_dep_helper(d.ins, dma_chain[-2].ins,
                                       reason="input dma ordering")
                    dma_chain.append(d)
                    all_q_tiles[h].append(xt)

            for h in range(2):
                oout = outs[h]
                q_tiles = all_q_tiles[h]
                for i in range(NBLK):
                    w_all = DA - P * i
                    chunks = []
                    off = 0
                    while off < w_all:
                        w = min(512, w_all - off)
                        chunks.append((off, w))
                        off += w
                    pts = [pp.tile([P, 512], mybir.dt.float32, tag="ps",
                                   name=f"ps{h}b{i}c{ci}")
                           for ci in range(len(chunks))]
                    for t in range(NB):
                        q, l = divmod(t, L)
                        xt = q_tiles[q]
                        base = l * DA + P * i
                        lhsT = xt[:, base:base + P]
                        for (off, w), pt in zip(chunks, pts):
                            nc.tensor.matmul(pt[:, :w], lhsT,
                                             xt[:, base + off:base + off + w],
                                             start=(t == 0), stop=(t == NB - 1))
                    ot = op.tile([P, w_all], mybir.dt.float32, tag="ot",
                                 name=f"o{h}b{i}")
                    for (off, w), pt in zip(chunks, pts):
                        nc.vector.tensor_copy(ot[:, off:off + w], pt[:, :w])
                    nc.sync.dma_start(out=oout[P * i:P * (i + 1), P * i:DA],
                                      in_=ot[:])
    nc.compile()
    return nc


def _get_nc():
    if "nc" not in _STATE:
        _STATE["nc"] = _build()
    return _STATE["nc"]


def _prep_half(xh):
    """xh: (128, 128, 768) f32 for one half -> per-core list of (NQ,P,L*DA) f16."""
    out = []
    for c in range(NCORES):
        blk = xh[NB * c:NB * (c + 1)]                      # (16, 128, 768)
        arr = np.zeros((NB, P, DA), dtype=np.float16)
        arr[:, :, :D] = blk
        for j in range(NB):
            arr[j, :, D + j] = 1.0
        # (t=4q+l, p, f) -> (q, p, l*DA+f)
        out.append(np.ascontiguousarray(
            arr.reshape(NQ, L, P, DA).transpose(0, 2, 1, 3).reshape(NQ, P, L * DA)))
    return out


def kernel(x, label=None, genre_label=None, _trace=False):
    from concourse.bass_utils import run_bass_kernel_spmd

    nc = _get_nc()

    x = np.asarray(x, dtype=np.float32)
    halves = [_prep_half(x[0::2]), _prep_half(x[1::2])]
    in_maps = [{"x0": halves[0][c], "x1": halves[1][c]} for c in range(NCORES)]

    res = run_bass_kernel_spmd(nc, in_maps, list(range(NCORES)), trace=_trace)
    LAST["res"] = res

    B = x.shape[0] // 2          # 128 b's per half
    N = x.shape[1]               # 128 rows per b
    tol = B * N

    loss = 0.0
    for h in range(2):
        U = np.zeros((D, D), dtype=np.float64)
        S = np.zeros((B, D), dtype=np.float64)
        for c in range(NCORES):
            o = np.asarray(res.results[c][f"o{h}"], dtype=np.float64)
            for i in range(NBLK):
                r = slice(P * i, P * (i + 1))
                U[r, P * i:D] += o[r, P * i:D]
            S[NB * c:NB * (c + 1)] += o[:, D:DA].T
        G = np.zeros((D, D), dtype=np.float64)
        for i in range(NBLK):
            ri = slice(P * i, P * (i + 1))
            G[ri, ri] = U[ri, ri]
            for j in range(i + 1, NBLK):
                rj = slice(P * j, P * (j + 1))
                G[ri, rj] = U[ri, rj]
                G[rj, ri] = U[ri, rj].T
        xbar = S / N
        M = xbar.T @ xbar
        mean = xbar.mean(axis=0)
        within = (G - N * M) / tol
        between = N * (M - B * np.outer(mean, mean)) / tol
        w_h = within / np.sqrt(np.sum(np.diagonal(within) ** 2))
        b_h = between / np.sqrt(np.sum(np.diagonal(between) ** 2))
        if h == 0:
            w0, b0 = w_h, b_h
        else:
            loss = np.sum((w0 - w_h) ** 2) + np.sum((b0 - b_h) ** 2)
    return np.asarray(loss, dtype=np.float32)


# revision 8
# speedup vs baseline: 1.5349x; 1.4811x over previous
"""Trainium2 Bass kernel for nn_LossFunction_40346922778857.

Computes: scatter-loss over x (256,128,768).
  x1 = x[::2], x2 = x[1::2]  (each (128,128,768))
  per half: within (D,D), between (D,D) scatter matrices, corr-normalized,
  loss = sum((w1-w2)^2) + sum((b1-b2)^2).

Strategy (data-parallel over b across 8 cores):
  within = (G - N * Xbar^T Xbar) / (B*N)   with G = X^T X over (B*N, D)
  between = N * (Xbar^T Xbar - B mean mean^T) / (B*N)
  Each core computes partial G (upper-triangle 128-row blocks, fp16 inputs,
  fp32 PSUM accumulation) for its 16 even + 16 odd b's.  Per-b row-sums S
  fall out of the same matmuls via 16 appended one-hot columns.
  Host sums the 8 partials and finishes the O(D^2) algebra.
"""

import numpy as np

P = 128          # partitions / rows per b
D = 768          # feature dim
NB = 16          # number of b's (tiles) per half per core
DA = D + NB      # augmented width (one-hot tile-index columns)
L = 4            # k-tiles per DMA quarter
NQ = NB // L     # quarters per half
NCORES = 8
NBLK = D // P    # 6 row blocks of G

_STATE = {}
LAST = {}
FP8 = True     # fp8e4 + DoubleRow tensor-engine path (rel err ~1e-4 vs ~5e-7 fp16)
ND = NB // 2   # double-k-tiles per half per core (DoubleRow contracts 256 rows)


def _chunks_for(w_all):
    chunks = []
    off = 0
    while off < w_all:
        w = min(512, w_all - off)
        chunks.append((off, w))
        off += w
    return chunks


def _build():
    import concourse.tile as tile
    from concourse import bacc, mybir
    from concourse.tile import add_dep_helper

    nc = bacc.Bacc("TRN2", target_bir_lowering=False, debug=False,
                   num_devices=NCORES)

    in_dt = mybir.dt.float8e4 if FP8 else mybir.dt.float16
    # fp8: quarter = 2 double-k-tiles, free layout (dt2, j, f); fp16: 4 k-tiles
    xins = [nc.dram_tensor(f"x{h}", [NQ, P, L * DA], in_dt,
                           kind="ExternalInput").ap() for h in range(2)]
    outs = [nc.dram_tensor(f"o{h}", [D, DA], mybir.dt.float32,
                           kind="ExternalOutput").ap() for h in range(2)]

    with tile.TileContext(nc) as tc:
        with tc.tile_pool(name="xp", bufs=2 * NQ) as xp, \
             tc.tile_pool(name="wp", bufs=1) as wp, \
             tc.tile_pool(name="pp", bufs=4, space="PSUM") as pp, \
             tc.tile_pool(name="wpp", bufs=1, space="PSUM") as wpp, \
             tc.tile_pool(name="op", bufs=3) as op:
            # PE warm-up: dummy matmuls while input DMAs stream, so the HAM
            # clock gate is at 8/8 when real matmuls start.
            wt = wp.tile([P, P], mybir.dt.float16, tag="wt")
            nc.vector.memset(wt[:], 0.0)
            wps = wpp.tile([P, P], mybir.dt.float32, tag="wps")
            for _ in range(24):
                nc.tensor.matmul(wps[:], wt[:], wt[:], start=True, stop=True)

            # Input DMAs, chained depth-2 so tiles arrive roughly in
            # consumption order instead of 8-way fair sharing.
            dma_chain = []
            all_q_tiles = [[], []]
            for h in range(2):
                xin = xins[h]
                for q in range(NQ):
                    if FP8:
                        xt = xp.tile([P, 2, 2, DA], in_dt, tag="xt",
                                     name=f"x{h}q{q}")
                        d = nc.sync.dma_start(
                            out=xt[:], in_=xin[q].rearrange("p (a b f) -> p a b f",
                                                            a=2, b=2))
                    else:
                        xt = xp.tile([P, L * DA], in_dt, tag="xt",
                                     name=f"x{h}q{q}")
                        d = nc.sync.dma_start(out=xt[:], in_=xin[q])
                    if len(dma_chain) >= 2:
                        add_dep_helper(d.ins, dma_chain[-2].ins,
                                       reason="input dma ordering")
                    dma_chain.append(d)
                    all_q_tiles[h].append(xt)

            for h in range(2):
                oout = outs[h]
                q_tiles = all_q_tiles[h]
                for i in range(NBLK):
                    w_all = DA - P * i
                    chunks = _chunks_for(w_all)
                    pts = [pp.tile([P, 512], mybir.dt.float32, tag="ps",
                                   name=f"ps{h}b{i}c{ci}")
                           for ci in range(len(chunks))]
                    if FP8:
                        for td in range(ND):
                            q, dt2 = divmod(td, 2)
                            xt = q_tiles[q]
                            c0 = P * i
                            lhsT = xt[:, dt2, :, c0:c0 + P]
                            for (off, w), pt in zip(chunks, pts):
                                nc.tensor.matmul(
                                    pt[:, :w], lhsT,
                                    xt[:, dt2, :, c0 + off:c0 + off + w],
                                    start=(td == 0), stop=(td == ND - 1),
                                    perf_mode=mybir.MatmulPerfMode.DoubleRow)
                    else:
                        for t in range(NB):
                            q, l = divmod(t, L)
                            xt = q_tiles[q]
                            base = l * DA + P * i
                            lhsT = xt[:, base:base + P]
                            for (off, w), pt in zip(chunks, pts):
                                nc.tensor.matmul(
                                    pt[:, :w], lhsT,
                                    xt[:, base + off:base + off + w],
                                    start=(t == 0), stop=(t == NB - 1))
                    ot = op.tile([P, w_all], mybir.dt.float32, tag="ot",
                                 name=f"o{h}b{i}")
                    for (off, w), pt in zip(chunks, pts):
                        nc.vector.tensor_copy(ot[:, off:off + w], pt[:, :w])
                    # scalar-engine HWDGE ring: output never queues behind input
                    nc.scalar.dma_start(out=oout[P * i:P * (i + 1), P * i:DA],
                                        in_=ot[:])
    nc.compile()
    return nc


def _get_nc():
    if "nc" not in _STATE:
        _STATE["nc"] = _build()
    return _STATE["nc"]


def _prep_half(xh):
    """xh: (128, 128, 768) f32 for one half -> per-core list of (NQ,P,L*DA)."""
    out = []
    for c in range(NCORES):
        blk = xh[NB * c:NB * (c + 1)]                      # (16, 128, 768)
        arr = np.zeros((NB, P, DA), dtype=np.float16)
        arr[:, :, :D] = blk
        for j in range(NB):
            arr[j, :, D + j] = 1.0
        if FP8:
            import ml_dtypes
            arr8 = arr.astype(ml_dtypes.float8_e4m3)
            # t = 4q + 2*dt2 + j -> (q, p, dt2, j, f)
            out.append(np.ascontiguousarray(
                arr8.reshape(NQ, 2, 2, P, DA).transpose(0, 3, 1, 2, 4)
                    .reshape(NQ, P, L * DA)))
        else:
            # (t=4q+l, p, f) -> (q, p, l*DA+f)
            out.append(np.ascontiguousarray(
                arr.reshape(NQ, L, P, DA).transpose(0, 2, 1, 3)
                   .reshape(NQ, P, L * DA)))
    return out


def kernel(x, label=None, genre_label=None, _trace=False):
    from concourse.bass_utils import run_bass_kernel_spmd

    nc = _get_nc()

    x = np.asarray(x, dtype=np.float32)
    halves = [_prep_half(x[0::2]), _prep_half(x[1::2])]
    in_maps = [{"x0": halves[0][c], "x1": halves[1][c]} for c in range(NCORES)]

    res = run_bass_kernel_spmd(nc, in_maps, list(range(NCORES)), trace=_trace)
    LAST["res"] = res

    B = x.shape[0] // 2          # 128 b's per half
    N = x.shape[1]               # 128 rows per b
    tol = B * N

    loss = 0.0
    for h in range(2):
        U = np.zeros((D, D), dtype=np.float64)
        S = np.zeros((B, D), dtype=np.float64)
        for c in range(NCORES):
            o = np.asarray(res.results[c][f"o{h}"], dtype=np.float64)
            for i in range(NBLK):
                r = slice(P * i, P * (i + 1))
                U[r, P * i:D] += o[r, P * i:D]
            S[NB * c:NB * (c + 1)] += o[:, D:DA].T
        G = np.zeros((D, D), dtype=np.float64)
        for i in range(NBLK):
            ri = slice(P * i, P * (i + 1))
            G[ri, ri] = U[ri, ri]
            for j in range(i + 1, NBLK):
                rj = slice(P * j, P * (j + 1))
                G[ri, rj] = U[ri, rj]
                G[rj, ri] = U[ri, rj].T
        xbar = S / N
        M = xbar.T @ xbar
        mean = xbar.mean(axis=0)
        within = (G - N * M) / tol
        between = N * (M - B * np.outer(mean, mean)) / tol
        w_h = within / np.sqrt(np.sum(np.diagonal(within) ** 2))
        b_h = between / np.sqrt(np.sum(np.diagonal(between) ** 2))
        if h == 0:
            w0, b0 = w_h, b_h
        else:
            loss = np.sum((w0 - w_h) ** 2) + np.sum((b0 - b_h) ** 2)
    return np.asarray(loss, dtype=np.float32)


# revision 11
# speedup vs baseline: 1.6557x; 1.0787x over previous
"""Trainium2 Bass kernel for nn_LossFunction_40346922778857.

Computes: scatter-loss over x (256,128,768).
  x1 = x[::2], x2 = x[1::2]  (each (128,128,768))
  per half: within (D,D), between (D,D) scatter matrices, corr-normalized,
  loss = sum((w1-w2)^2) + sum((b1-b2)^2).

Strategy (data-parallel over b across 8 cores):
  within = (G - N * Xbar^T Xbar) / (B*N)   with G = X^T X over (B*N, D)
  between = N * (Xbar^T Xbar - B mean mean^T) / (B*N)
  Each core computes partial G (upper-triangle 128-row blocks, fp16 inputs,
  fp32 PSUM accumulation) for its 16 even + 16 odd b's.  Per-b row-sums S
  fall out of the same matmuls via 16 appended one-hot columns.
  Host sums the 8 partials and finishes the O(D^2) algebra.
"""

import numpy as np

P = 128          # partitions / rows per b
D = 768          # feature dim
NB = 16          # number of b's (tiles) per half per core
DA = D + NB      # augmented width (one-hot tile-index columns)
L = 4            # k-tiles per DMA quarter
NQ = NB // L     # quarters per half
NCORES = 8
NBLK = D // P    # 6 row blocks of G

_STATE = {}
LAST = {}
FP8 = True     # fp8e4 + DoubleRow tensor-engine path (rel err ~1e-4 vs ~5e-7 fp16)
ND = NB // 2   # double-k-tiles per half per core (DoubleRow contracts 256 rows)


def _chunks_for(w_all):
    chunks = []
    off = 0
    while off < w_all:
        w = min(512, w_all - off)
        chunks.append((off, w))
        off += w
    return chunks


def _build():
    import concourse.tile as tile
    from concourse import bacc, mybir
    from concourse.tile import add_dep_helper

    nc = bacc.Bacc("TRN2", target_bir_lowering=False, debug=False,
                   num_devices=NCORES)

    in_dt = mybir.dt.float8e4 if FP8 else mybir.dt.float16
    # fp8: quarter = 2 double-k-tiles, free layout (dt2, j, f); fp16: 4 k-tiles
    xins = [nc.dram_tensor(f"x{h}", [NQ, P, L * DA], in_dt,
                           kind="ExternalInput").ap() for h in range(2)]
    outs = [nc.dram_tensor(f"o{h}", [D, DA], mybir.dt.bfloat16,
                           kind="ExternalOutput").ap() for h in range(2)]

    with tile.TileContext(nc) as tc:
        with tc.tile_pool(name="xp", bufs=2 * NQ) as xp, \
             tc.tile_pool(name="wp", bufs=1) as wp, \
             tc.tile_pool(name="pp", bufs=6, space="PSUM") as pp, \
             tc.tile_pool(name="wpp", bufs=1, space="PSUM") as wpp, \
             tc.tile_pool(name="op", bufs=6) as op:
            # PE warm-up: dummy matmuls while input DMAs stream, so the HAM
            # clock gate is at 8/8 when real matmuls start.
            wt = wp.tile([P, P], mybir.dt.float16, tag="wt")
            nc.vector.memset(wt[:], 0.0)
            wps = wpp.tile([P, P], mybir.dt.float32, tag="wps")
            for _ in range(22):
                nc.tensor.matmul(wps[:], wt[:], wt[:], start=True, stop=True)

            # Input DMAs, chained depth-2 so tiles arrive roughly in
            # consumption order instead of 8-way fair sharing.
            dma_chain = []
            all_q_tiles = [[], []]
            for h in range(2):
                xin = xins[h]
                for q in range(NQ):
                    if FP8:
                        xt = xp.tile([P, 2, 2, DA], in_dt, tag="xt",
                                     name=f"x{h}q{q}")
                        d = nc.sync.dma_start(
                            out=xt[:], in_=xin[q].rearrange("p (a b f) -> p a b f",
                                                            a=2, b=2))
                    else:
                        xt = xp.tile([P, L * DA], in_dt, tag="xt",
                                     name=f"x{h}q{q}")
                        d = nc.sync.dma_start(out=xt[:], in_=xin[q])
                    if len(dma_chain) >= 2:
                        add_dep_helper(d.ins, dma_chain[-2].ins,
                                       reason="input dma ordering")
                    dma_chain.append(d)
                    all_q_tiles[h].append(xt)

            last_in = dma_chain[-1]
            # Two sweeps of row-blocks per half, k-tile-outer within a sweep:
            # one arrived quarter unlocks ALL its row-block matmuls (~2.9us of
            # PE work per quarter vs ~1.2us stream time -> no input starvation).
            for h in range(2):
                oout = outs[h]
                q_tiles = all_q_tiles[h]
                for sweep in ((0, 1, 2), (3, 4, 5)):
                    pts = {}
                    for i in sweep:
                        for ci in range(len(_chunks_for(DA - P * i))):
                            pts[i, ci] = pp.tile([P, 512], mybir.dt.float32,
                                                 tag="ps", name=f"ps{h}b{i}c{ci}")
                    if FP8:
                        for td in range(ND):
                            q, dt2 = divmod(td, 2)
                            xt = q_tiles[q]
                            for i in sweep:
                                c0 = P * i
                                lhsT = xt[:, dt2, :, c0:c0 + P]
                                for ci, (off, w) in enumerate(_chunks_for(DA - c0)):
                                    nc.tensor.matmul(
                                        pts[i, ci][:, :w], lhsT,
                                        xt[:, dt2, :, c0 + off:c0 + off + w],
                                        start=(td == 0), stop=(td == ND - 1),
                                        perf_mode=mybir.MatmulPerfMode.DoubleRow)
                    else:
                        for t in range(NB):
                            q, l = divmod(t, L)
                            xt = q_tiles[q]
                            for i in sweep:
                                base = l * DA + P * i
                                lhsT = xt[:, base:base + P]
                                for ci, (off, w) in enumerate(_chunks_for(DA - P * i)):
                                    nc.tensor.matmul(
                                        pts[i, ci][:, :w], lhsT,
                                        xt[:, base + off:base + off + w],
                                        start=(t == 0), stop=(t == NB - 1))
                    for i in sweep:
                        w_all = DA - P * i
                        ot = op.tile([P, w_all], mybir.dt.bfloat16, tag="ot",
                                     name=f"o{h}b{i}")
                        for ci, (off, w) in enumerate(_chunks_for(w_all)):
                            nc.vector.tensor_copy(ot[:, off:off + w],
                                                  pts[i, ci][:, :w])
                        # scalar-engine HWDGE ring + gated behind input so
                        # output traffic never steals input bandwidth
                        dout = nc.scalar.dma_start(
                            out=oout[P * i:P * (i + 1), P * i:DA], in_=ot[:])
                        add_dep_helper(dout.ins, last_in.ins,
                                       reason="outputs after inputs")
    nc.compile()
    return nc


def _get_nc():
    if "nc" not in _STATE:
        _STATE["nc"] = _build()
    return _STATE["nc"]


def _prep_half(xh):
    """xh: (128, 128, 768) f32 for one half -> per-core list of (NQ,P,L*DA)."""
    out = []
    for c in range(NCORES):
        blk = xh[NB * c:NB * (c + 1)]                      # (16, 128, 768)
        arr = np.zeros((NB, P, DA), dtype=np.float16)
        arr[:, :, :D] = blk
        for j in range(NB):
            arr[j, :, D + j] = 1.0
        if FP8:
            import ml_dtypes
            arr8 = arr.astype(ml_dtypes.float8_e4m3)
            # t = 4q + 2*dt2 + j -> (q, p, dt2, j, f)
            out.append(np.ascontiguousarray(
                arr8.reshape(NQ, 2, 2, P, DA).transpose(0, 3, 1, 2, 4)
                    .reshape(NQ, P, L * DA)))
        else:
            # (t=4q+l, p, f) -> (q, p, l*DA+f)
            out.append(np.ascontiguousarray(
                arr.reshape(NQ, L, P, DA).transpose(0, 2, 1, 3)
                   .reshape(NQ, P, L * DA)))
    return out


def kernel(x, label=None, genre_label=None, _trace=False):
    from concourse.bass_utils import run_bass_kernel_spmd

    nc = _get_nc()

    x = np.asarray(x, dtype=np.float32)
    halves = [_prep_half(x[0::2]), _prep_half(x[1::2])]
    in_maps = [{"x0": halves[0][c], "x1": halves[1][c]} for c in range(NCORES)]

    res = run_bass_kernel_spmd(nc, in_maps, list(range(NCORES)), trace=_trace)
    LAST["res"] = res

    B = x.shape[0] // 2          # 128 b's per half
    N = x.shape[1]               # 128 rows per b
    tol = B * N

    loss = 0.0
    for h in range(2):
        U = np.zeros((D, D), dtype=np.float64)
        S = np.zeros((B, D), dtype=np.float64)
        for c in range(NCORES):
            o = np.asarray(res.results[c][f"o{h}"], dtype=np.float64)
            for i in range(NBLK):
                r = slice(P * i, P * (i + 1))
                U[r, P * i:D] += o[r, P * i:D]
            S[NB * c:NB * (c + 1)] += o[:, D:DA].T
        G = np.zeros((D, D), dtype=np.float64)
        for i in range(NBLK):
            ri = slice(P * i, P * (i + 1))
            G[ri, ri] = U[ri, ri]
            for j in range(i + 1, NBLK):
                rj = slice(P * j, P * (j + 1))
                G[ri, rj] = U[ri, rj]
                G[rj, ri] = U[ri, rj].T
        xbar = S / N
        M = xbar.T @ xbar
        mean = xbar.mean(axis=0)
        within = (G - N * M) / tol
        between = N * (M - B * np.outer(mean, mean)) / tol
        w_h = within / np.sqrt(np.sum(np.diagonal(within) ** 2))
        b_h = between / np.sqrt(np.sum(np.diagonal(between) ** 2))
        if h == 0:
            w0, b0 = w_h, b_h
        else:
            loss = np.sum((w0 - w_h) ** 2) + np.sum((b0 - b_h) ** 2)
    return np.asarray(loss, dtype=np.float32)


# revision 15
# speedup vs baseline: 1.6710x; 1.0092x over previous
"""Trainium2 Bass kernel for nn_LossFunction_40346922778857.

Computes: scatter-loss over x (256,128,768).
  x1 = x[::2], x2 = x[1::2]  (each (128,128,768))
  per half: within (D,D), between (D,D) scatter matrices, corr-normalized,
  loss = sum((w1-w2)^2) + sum((b1-b2)^2).

Strategy (data-parallel over b across 8 cores):
  within = (G - N * Xbar^T Xbar) / (B*N)   with G = X^T X over (B*N, D)
  between = N * (Xbar^T Xbar - B mean mean^T) / (B*N)
  Each core computes partial G (upper-triangle 128-row blocks, fp16 inputs,
  fp32 PSUM accumulation) for its 16 even + 16 odd b's.  Per-b row-sums S
  fall out of the same matmuls via 16 appended one-hot columns.
  Host sums the 8 partials and finishes the O(D^2) algebra.
"""

import numpy as np

P = 128          # partitions / rows per b
D = 768          # feature dim
NB = 16          # number of b's (tiles) per half per core
DA = D + NB      # augmented width (one-hot tile-index columns)
L = 4            # k-tiles per DMA quarter
NQ = NB // L     # quarters per half
NCORES = 8
NBLK = D // P    # 6 row blocks of G

_STATE = {}
LAST = {}
FP8 = True     # fp8e4 + DoubleRow tensor-engine path (rel err ~1e-4 vs ~5e-7 fp16)
ND = NB // 2   # double-k-tiles per half per core (DoubleRow contracts 256 rows)


def _chunks_for(w_all):
    chunks = []
    off = 0
    while off < w_all:
        w = min(512, w_all - off)
        chunks.append((off, w))
        off += w
    return chunks


def _build():
    import concourse.tile as tile
    from concourse import bacc, mybir
    from concourse.tile import add_dep_helper

    nc = bacc.Bacc("TRN2", target_bir_lowering=False, debug=False,
                   num_devices=NCORES)

    in_dt = mybir.dt.float8e4 if FP8 else mybir.dt.float16
    # fp8: quarter = 2 double-k-tiles, free layout (dt2, j, f); fp16: 4 k-tiles
    xins = [nc.dram_tensor(f"x{h}", [NQ, P, L * DA], in_dt,
                           kind="ExternalInput").ap() for h in range(2)]
    outs = [nc.dram_tensor(f"o{h}", [D, DA], mybir.dt.bfloat16,
                           kind="ExternalOutput").ap() for h in range(2)]

    with tile.TileContext(nc) as tc:
        with tc.tile_pool(name="xp", bufs=2 * NQ) as xp, \
             tc.tile_pool(name="wp", bufs=1) as wp, \
             tc.tile_pool(name="pp", bufs=6, space="PSUM") as pp, \
             tc.tile_pool(name="wpp", bufs=1, space="PSUM") as wpp, \
             tc.tile_pool(name="op", bufs=6) as op:
            # PE warm-up: dummy matmuls while input DMAs stream, so the HAM
            # clock gate is at 8/8 when real matmuls start.
            wt = wp.tile([P, P], mybir.dt.float16, tag="wt")
            nc.vector.memset(wt[:], 0.0)
            wps = wpp.tile([P, P], mybir.dt.float32, tag="wps")
            for _ in range(16):
                nc.tensor.matmul(wps[:], wt[:], wt[:], start=True, stop=True)

            # Input DMAs, chained depth-2 so tiles arrive roughly in
            # consumption order instead of 8-way fair sharing.
            dma_chain = []
            all_q_tiles = [[], []]
            for h in range(2):
                xin = xins[h]
                for q in range(NQ):
                    if FP8:
                        xt = xp.tile([P, 2, 2, DA], in_dt, tag="xt",
                                     name=f"x{h}q{q}")
                        d = nc.sync.dma_start(
                            out=xt[:], in_=xin[q].rearrange("p (a b f) -> p a b f",
                                                            a=2, b=2))
                    else:
                        xt = xp.tile([P, L * DA], in_dt, tag="xt",
                                     name=f"x{h}q{q}")
                        d = nc.sync.dma_start(out=xt[:], in_=xin[q])
                    if len(dma_chain) == 1:
                        # first link depth-1: q0 streams alone at full BW
                        add_dep_helper(d.ins, dma_chain[-1].ins,
                                       reason="input dma ordering")
                    elif len(dma_chain) >= 2:
                        add_dep_helper(d.ins, dma_chain[-2].ins,
                                       reason="input dma ordering")
                    dma_chain.append(d)
                    all_q_tiles[h].append(xt)

            last_in = dma_chain[-1]
            # Two sweeps of row-blocks per half, k-tile-outer within a sweep:
            # one arrived quarter unlocks ALL its row-block matmuls (~2.9us of
            # PE work per quarter vs ~1.2us stream time -> no input starvation).
            for h in range(2):
                oout = outs[h]
                q_tiles = all_q_tiles[h]
                # h0 first sweep is k-outer across 3 row-blocks (input still
                # streaming); once data is resident, per-block passes retire
                # PSUM + outputs sooner.
                sweeps = (((0, 1, 2), (3,), (4,), (5,)) if h == 0 else
                          ((0,), (1,), (2,), (3,), (4,), (5,)))
                for sweep in sweeps:
                    pts = {}
                    for i in sweep:
                        for ci in range(len(_chunks_for(DA - P * i))):
                            pts[i, ci] = pp.tile([P, 512], mybir.dt.float32,
                                                 tag="ps", name=f"ps{h}b{i}c{ci}")
                    if FP8:
                        for td in range(ND):
                            q, dt2 = divmod(td, 2)
                            xt = q_tiles[q]
                            for i in sweep:
                                c0 = P * i
                                lhsT = xt[:, dt2, :, c0:c0 + P]
                                for ci, (off, w) in enumerate(_chunks_for(DA - c0)):
                                    nc.tensor.matmul(
                                        pts[i, ci][:, :w], lhsT,
                                        xt[:, dt2, :, c0 + off:c0 + off + w],
                                        start=(td == 0), stop=(td == ND - 1),
                                        perf_mode=mybir.MatmulPerfMode.DoubleRow)
                    else:
                        for t in range(NB):
                            q, l = divmod(t, L)
                            xt = q_tiles[q]
                            for i in sweep:
                                base = l * DA + P * i
                                lhsT = xt[:, base:base + P]
                                for ci, (off, w) in enumerate(_chunks_for(DA - P * i)):
                                    nc.tensor.matmul(
                                        pts[i, ci][:, :w], lhsT,
                                        xt[:, base + off:base + off + w],
                                        start=(t == 0), stop=(t == NB - 1))
                    for i in sweep:
                        w_all = DA - P * i
                        ot = op.tile([P, w_all], mybir.dt.bfloat16, tag="ot",
                                     name=f"o{h}b{i}")
                        for ci, (off, w) in enumerate(_chunks_for(w_all)):
                            nc.vector.tensor_copy(ot[:, off:off + w],
                                                  pts[i, ci][:, :w])
                        # scalar-engine HWDGE ring + gated behind input so
                        # output traffic never steals input bandwidth
                        dout = nc.scalar.dma_start(
                            out=oout[P * i:P * (i + 1), P * i:DA], in_=ot[:])
                        add_dep_helper(dout.ins, last_in.ins,
                                       reason="outputs after inputs")
    nc.compile()
    return nc


def _get_nc():
    if "nc" not in _STATE:
        _STATE["nc"] = _build()
    return _STATE["nc"]


def _prep_half(xh):
    """xh: (128, 128, 768) f32 for one half -> per-core list of (NQ,P,L*DA)."""
    out = []
    for c in range(NCORES):
        blk = xh[NB * c:NB * (c + 1)]                      # (16, 128, 768)
        arr = np.zeros((NB, P, DA), dtype=np.float16)
        arr[:, :, :D] = blk
        for j in range(NB):
            arr[j, :, D + j] = 1.0
        if FP8:
            import ml_dtypes
            arr8 = arr.astype(ml_dtypes.float8_e4m3)
            # t = 4q + 2*dt2 + j -> (q, p, dt2, j, f)
            out.append(np.ascontiguousarray(
                arr8.reshape(NQ, 2, 2, P, DA).transpose(0, 3, 1, 2, 4)
                    .reshape(NQ, P, L * DA)))
        else:
            # (t=4q+l, p, f) -> (q, p, l*DA+f)
            out.append(np.ascontiguousarray(
                arr.reshape(NQ, L, P, DA).transpose(0, 2, 1, 3)
                   .reshape(NQ, P, L * DA)))
    return out


def kernel(x, label=None, genre_label=None, _trace=False):
    from concourse.bass_utils import run_bass_kernel_spmd

    nc = _get_nc()

    x = np.asarray(x, dtype=np.float32)
    halves = [_prep_half(x[0::2]), _prep_half(x[1::2])]
    in_maps = [{"x0": halves[0][c], "x1": halves[1][c]} for c in range(NCORES)]

    # First execution of a freshly compiled NEFF has been observed to be
    # flaky (garbage output or device error); validate and retry.
    res = None
    for attempt in range(3):
        try:
            res = run_bass_kernel_spmd(nc, in_maps, list(range(NCORES)),
                                       trace=_trace)
        except Exception:
            if attempt == 2:
                raise
            continue
        ok = all(
            np.isfinite(np.asarray(res.results[c][f"o{h}"],
                                   dtype=np.float32)).all()
            and np.any(np.asarray(res.results[c][f"o{h}"], dtype=np.float32))
            for c in range(NCORES) for h in range(2))
        if ok:
            break
    LAST["res"] = res

    B = x.shape[0] // 2          # 128 b's per half
    N = x.shape[1]               # 128 rows per b
    tol = B * N

    loss = 0.0
    for h in range(2):
        U = np.zeros((D, D), dtype=np.float64)
        S = np.zeros((B, D), dtype=np.float64)
        for c in range(NCORES):
            o = np.asarray(res.results[c][f"o{h}"], dtype=np.float64)
            for i in range(NBLK):
                r = slice(P * i, P * (i + 1))
                U[r, P * i:D] += o[r, P * i:D]
            S[NB * c:NB * (c + 1)] += o[:, D:DA].T
        G = np.zeros((D, D), dtype=np.float64)
        for i in range(NBLK):
            ri = slice(P * i, P * (i + 1))
            G[ri, ri] = U[ri, ri]
            for j in range(i + 1, NBLK):
                rj = slice(P * j, P * (j + 1))
                G[ri, rj] = U[ri, rj]
                G[rj, ri] = U[ri, rj].T
        xbar = S / N
        M = xbar.T @ xbar
        mean = xbar.mean(axis=0)
        within = (G - N * M) / tol
        between = N * (M - B * np.outer(mean, mean)) / tol
        w_h = within / np.sqrt(np.sum(np.diagonal(within) ** 2))
        b_h = between / np.sqrt(np.sum(np.diagonal(between) ** 2))
        if h == 0:
            w0, b0 = w_h, b_h
        else:
            loss = np.sum((w0 - w_h) ** 2) + np.sum((b0 - b_h) ** 2)
    return np.asarray(loss, dtype=np.float32)
